# revision 19
# baseline (speedup 1.0000x reference)
"""Trainium2 Bass kernel for nn_DecoderRNN (LSTM decoder + vocab projection).

Strategy (8 NeuronCores, SPMD):
  - Recurrence tensor-parallel: core d owns h-dims [128d, 128(d+1)). Each step it
    computes its 4x128 gate rows (transposed layout [gate_dim, batch]) with bf16
    matmuls accumulating in PSUM, applies sigmoid/tanh on ScalarE (bias fused),
    updates c/h on VectorE, and AllGathers the bf16 h^T shard ([128,64] per rank
    -> [1024,64]) so every core has the full hidden state for the next step.
  - fc projection vocab-parallel: core d owns 6400 vocab rows (V padded to 51200).
    W_fc^T (bf16) is SBUF-resident. Steps are batched in pairs so the stationary
    operand is [128k, 128m] (m = 2 steps x 64 batch) at full PE utilization,
    streaming W_fc^T in N=512 tiles. b_fc is added during the PSUM->SBUF drain
    on VectorE from an SBUF-resident [128, 6400] fp32 tile built once via K=1
    ones-matmuls.
  - x_proj = features @ W_ih^T is step-invariant: computed once on device, stored
    fp32, and injected into each step's PSUM group via an fp32 identity matmul.
"""

import numpy as np
import ml_dtypes

import concourse.bass as bass
import concourse.bacc as bacc
import concourse.tile as tile
import concourse.mybir as mybir
from concourse import bass_utils
from concourse.bass_interp import get_hw_module

BF = ml_dtypes.bfloat16
FP32 = mybir.dt.float32
BF16 = mybir.dt.bfloat16
AF = mybir.ActivationFunctionType

R = 8            # cores
B = 64           # batch
E = 512          # embed
H = 1024         # hidden
V = 50257        # vocab
VP = 51200       # padded vocab (multiple of 8*128)
VS = VP // R     # per-core vocab shard = 6400
NKH = H // 128   # 8 k-chunks over hidden
NKE = E // 128   # 4 k-chunks over embed
FC_N = 512       # fc free-dim tile
NFC = (VS + FC_N - 1) // FC_N  # 13 n-tiles (12x512 + 1x256)


def _emit(nc, tc, T, reps, wfcT, whhT, wihT, xT, bfc, bias, ident, out):
    NP = (T + 1) // 2  # step pairs
    with (
        tc.tile_pool(name="wpool", bufs=1) as wpool,
        tc.tile_pool(name="hpool", bufs=1) as hpool,
        tc.tile_pool(name="spool", bufs=3) as spool,
        tc.tile_pool(name="stage", bufs=6) as stage,
        tc.tile_pool(name="pg", bufs=1, space="PSUM") as pg_pool,
        tc.tile_pool(name="pf", bufs=4, space="PSUM") as pf_pool,
        tc.tile_pool(name="dram", bufs=2, space="DRAM") as dram,
    ):
        # ---- resident weights / constants (small DMAs first) ----
        wih_sb = wpool.tile([128, NKE, 512], BF16, name="wih_sb")
        nc.sync.dma_start(wih_sb[:], wihT.rearrange("(k p) n -> p k n", p=128))
        x_sb = wpool.tile([128, NKE, B], BF16, name="x_sb")
        nc.sync.dma_start(x_sb[:], xT.rearrange("(k p) n -> p k n", p=128))
        whh_sb = wpool.tile([128, NKH, 512], BF16, name="whh_sb")
        nc.sync.dma_start(whh_sb[:], whhT.rearrange("(k p) n -> p k n", p=128))
        bias_sb = wpool.tile([128, 4], FP32, name="bias_sb")
        nc.sync.dma_start(bias_sb[:], bias)
        id_sb = wpool.tile([128, 128], BF16, name="id_sb")
        nc.sync.dma_start(id_sb[:], ident)
        bfc_row = wpool.tile([1, VS], BF16, name="bfc_row")
        nc.sync.dma_start(bfc_row[:], bfc)
        ones_sb = wpool.tile([1, 128], BF16, name="ones_sb")
        nc.vector.memset(ones_sb[:], 1.0)
        # big W_fc^T load split by n-tile; DMAs are emitted inside the early
        # step slots so they don't delay the step-critical agi/hall DMAs
        wfc_sb = wpool.tile([128, NKH, VS], BF16, name="wfc_sb")
        wfcT_pkn = wfcT.rearrange("(k p) n -> p k n", p=128)

        def wfc_load(n_lo, n_hi):
            # issued from ScalarE (SWDGE) so the stream is not blocked behind
            # the Sync queue's hall-DMA wait on the first AllGather
            for n in range(n_lo, n_hi):
                nsz = min(FC_N, VS - n * FC_N)
                nc.scalar.dma_start(
                    wfc_sb[:, :, n * FC_N : n * FC_N + nsz],
                    wfcT_pkn[:, :, n * FC_N : n * FC_N + nsz],
                )

        bfc_sb = wpool.tile([128, VS], FP32, name="bfc_sb")

        # ---- x_proj^T [4 gate tiles][128, B], bf16 ----
        xproj_sb = wpool.tile([128, 4, B], BF16, name="xproj_sb")
        for m in range(4):
            px = pg_pool.tile([128, B], FP32, name="px", tag="pg0")
            for k in range(NKE):
                nc.tensor.matmul(
                    px[:], wih_sb[:, k, 128 * m : 128 * (m + 1)], x_sb[:, k, :],
                    start=(k == 0), stop=(k == NKE - 1),
                )
            nc.scalar.copy(xproj_sb[:, m, :], px[:])

        # persistent state
        hall = hpool.tile([128, NP, NKH, 2 * B], BF16, name="hall")
        cT = hpool.tile([128, B], FP32, name="cT")

        def fc_emit(p, n_lo, n_hi, msz):
            first_mm, last_mm = None, None
            for n in range(n_lo, n_hi):
                nsz = min(FC_N, VS - n * FC_N)
                pf = pf_pool.tile([128, FC_N], FP32, name="pf", tag="pf")
                for k in range(NKH):
                    i = nc.tensor.matmul(
                        pf[:msz, :nsz],
                        hall[:, p, k, :msz],
                        wfc_sb[:, k, n * FC_N : n * FC_N + nsz],
                        start=(k == 0), stop=(k == NKH - 1),
                    )
                    if first_mm is None:
                        first_mm = i
                    last_mm = i
                st = stage.tile([128, FC_N], FP32, name="st")
                nc.vector.tensor_add(
                    st[:msz, :nsz], pf[:msz, :nsz],
                    bfc_sb[:msz, n * FC_N : n * FC_N + nsz],
                )
                nc.sync.dma_start(
                    out[p, :msz, n * FC_N : n * FC_N + nsz], st[:msz, :nsz]
                )
            return first_mm, last_mm

        FC_SPLIT = 7  # n-tiles in the first half-chunk of a pair

        for rep in range(reps):
            nc.vector.memset(cT[:], 0.0)
            prev_filler_last = None
            for t in range(T):
                p, s = divmod(t, 2)
                # ---- gates^T in PSUM: 4 separate per-gate tiles [128, B] ----
                pg = [pg_pool.tile([128, B], FP32, name=f"pg{m}", tag=f"pg{m}") for m in range(4)]
                first_gates, last_gates = None, None
                for m in range(4):
                    i = nc.tensor.matmul(
                        pg[m][:], id_sb[:], xproj_sb[:, m, :],
                        start=True, stop=(t == 0),
                    )
                    if first_gates is None:
                        first_gates = i
                    last_gates = i
                    if t > 0:
                        pp, ss = divmod(t - 1, 2)
                        for k in range(NKH):
                            last_gates = nc.tensor.matmul(
                                pg[m][:],
                                whh_sb[:, k, 128 * m : 128 * (m + 1)],
                                hall[:, pp, k, B * ss : B * (ss + 1)],
                                start=False, stop=(k == NKH - 1),
                            )
                # keep PE stream interleaved: this slot's gates run after the
                # previous slot's PE filler work
                if prev_filler_last is not None:
                    tile.add_dep_helper(
                        first_gates.ins, prev_filler_last.ins, sync=False,
                        reason="slot order: gates after previous slot's fc filler",
                    )
                # ---- activations + c/h update (ordered for short critical path) ----
                i_s = spool.tile([128, B], FP32, name="i_s")
                f_s = spool.tile([128, B], FP32, name="f_s")
                g_t = spool.tile([128, B], FP32, name="g_t")
                o_s = spool.tile([128, B], FP32, name="o_s")
                t1 = spool.tile([128, B], FP32, name="t1")
                t2 = spool.tile([128, B], FP32, name="t2")
                tc_t = spool.tile([128, B], FP32, name="tc_t")
                h_bf = spool.tile([128, B], BF16, name="h_bf")
                nc.scalar.activation(i_s[:], pg[0][:], AF.Sigmoid, bias=bias_sb[:, 0:1])
                nc.scalar.activation(f_s[:], pg[1][:], AF.Sigmoid, bias=bias_sb[:, 1:2])
                nc.vector.tensor_mul(t2[:], f_s[:], cT[:])
                nc.scalar.activation(g_t[:], pg[2][:], AF.Tanh, bias=bias_sb[:, 2:3])
                nc.vector.tensor_mul(t1[:], i_s[:], g_t[:])
                nc.vector.tensor_add(cT[:], t1[:], t2[:])
                nc.scalar.activation(o_s[:], pg[3][:], AF.Sigmoid, bias=bias_sb[:, 3:4])
                nc.scalar.activation(tc_t[:], cT[:], AF.Tanh)
                nc.vector.tensor_mul(h_bf[:], o_s[:], tc_t[:])
                # ---- AllGather h^T shard -> full h^T (high priority: these
                # DMAs must not queue behind fc output stores) ----
                agi = dram.tile([128, B], BF16, name=f"agi{t}", tag=f"agi{t}")
                ago = dram.tile([H, B], BF16, name=f"ago{t}", tag=f"ago{t}")
                ago_pkn = ago.rearrange("(k p) n -> p k n", p=128)
                nc.sync.dma_start(agi[:], h_bf[:])
                nc.gpsimd.collective_compute(
                    "AllGather",
                    mybir.AluOpType.bypass,
                    ins=[agi.opt()],
                    outs=[ago.opt()],
                    replica_groups=[list(range(R))],
                )
                # split so the first gate matmuls can start on the first half
                nc.sync.dma_start(
                    hall[:, p, 0:4, B * s : B * (s + 1)], ago_pkn[:, 0:4, :]
                )
                nc.sync.dma_start(
                    hall[:, p, 4:8, B * s : B * (s + 1)], ago_pkn[:, 4:8, :]
                )
                # W_fc^T stream: spread over the first slots, after this
                # slot's critical DMAs
                if rep == 0 and t == 0:
                    # the whole W_fc^T stream fits inside the first AllGather's
                    # launch-skew wait
                    wfc_load(0, NFC)
                # ---- PE filler for this slot's AG flight ----
                first_fill, last_fill = None, None
                if t == 0 and rep == 0:
                    # b_fc broadcast via K=1 ones-matmuls fills slot 0
                    for n in range(NFC):
                        nsz = min(FC_N, VS - n * FC_N)
                        pb = pf_pool.tile([128, FC_N], FP32, name="pb", tag="pf")
                        i = nc.tensor.matmul(
                            pb[:, :nsz], ones_sb[:], bfc_row[:, n * FC_N : n * FC_N + nsz],
                            start=True, stop=True,
                        )
                        if first_fill is None:
                            first_fill = i
                        last_fill = i
                        nc.vector.tensor_copy(
                            bfc_sb[:, n * FC_N : n * FC_N + nsz], pb[:, :nsz]
                        )
                elif t >= 2:
                    # fc chunk for a pair whose data landed at step t-1
                    q, half = divmod(t - 2, 2)
                    if half == 0:
                        first_fill, last_fill = fc_emit(q, 0, FC_SPLIT, 2 * B)
                    else:
                        first_fill, last_fill = fc_emit(q, FC_SPLIT, NFC, 2 * B)
                if first_fill is not None:
                    tile.add_dep_helper(
                        first_fill.ins, last_gates.ins, sync=False,
                        reason="slot order: fc filler after this slot's gates",
                    )
                    prev_filler_last = last_fill
                else:
                    prev_filler_last = last_gates
            # ---- tail: last pair(s) not covered in-loop ----
            fc_emit(NP - 1, 0, NFC, B if (T % 2) else 2 * B)


def build(T, reps=1):
    nc = bacc.Bacc(
        "TRN2",
        target_bir_lowering=False,
        debug=False,
        enable_asserts=False,
        num_devices=R,
    )
    NP = (T + 1) // 2
    wfcT = nc.dram_tensor("wfcT", [H, VS], BF16, kind="ExternalInput").ap()
    whhT = nc.dram_tensor("whhT", [H, 512], BF16, kind="ExternalInput").ap()
    wihT = nc.dram_tensor("wihT", [E, 512], BF16, kind="ExternalInput").ap()
    xT = nc.dram_tensor("xT", [E, B], BF16, kind="ExternalInput").ap()
    bfc = nc.dram_tensor("bfc", [1, VS], BF16, kind="ExternalInput").ap()
    bias = nc.dram_tensor("bias", [128, 4], FP32, kind="ExternalInput").ap()
    ident = nc.dram_tensor("ident", [128, 128], BF16, kind="ExternalInput").ap()
    out = nc.dram_tensor("out", [NP, 128, VS], FP32, kind="ExternalOutput").ap()

    with tile.TileContext(nc) as tc:
        _emit(nc, tc, T, reps, wfcT, whhT, wihT, xT, bfc, bias, ident, out)
    nc.compile()
    nc.m = get_hw_module(nc.m)
    return nc


_NC_CACHE = {}


def get_nc(T, reps=1):
    key = (T, reps)
    if key not in _NC_CACHE:
        _NC_CACHE[key] = build(T, reps)
    return _NC_CACHE[key]


def make_in_maps(features, W_ih, W_hh, b_ih, b_hh, W_fc, b_fc):
    features = np.asarray(features, np.float32)
    W_ih = np.asarray(W_ih, np.float32)
    W_hh = np.asarray(W_hh, np.float32)
    W_fc = np.asarray(W_fc, np.float32)
    b = np.asarray(b_ih, np.float32) + np.asarray(b_hh, np.float32)
    b_fc = np.asarray(b_fc, np.float32)

    xT_np = np.ascontiguousarray(features.T).astype(BF)
    ident_np = np.eye(128, dtype=np.float32).astype(BF)
    W_fc_pad = np.zeros((VP, H), np.float32)
    W_fc_pad[:V] = W_fc
    bfc_pad = np.zeros((VP,), np.float32)
    bfc_pad[:V] = b_fc

    in_maps = []
    for d in range(R):
        gsel = np.concatenate(
            [np.arange(g * H + d * 128, g * H + (d + 1) * 128) for g in range(4)]
        )
        whhT_np = np.ascontiguousarray(W_hh[gsel].T).astype(BF)
        wihT_np = np.ascontiguousarray(W_ih[gsel].T).astype(BF)
        bias_np = np.ascontiguousarray(b[gsel].reshape(4, 128).T)
        wfcT_np = np.ascontiguousarray(W_fc_pad[d * VS : (d + 1) * VS].T).astype(BF)
        bfc_np = bfc_pad[d * VS : (d + 1) * VS].reshape(1, VS).astype(BF)
        in_maps.append(
            {
                "wfcT": wfcT_np,
                "whhT": whhT_np,
                "wihT": wihT_np,
                "xT": xT_np,
                "bfc": bfc_np,
                "bias": bias_np,
                "ident": ident_np,
            }
        )
    return in_maps


def assemble(results, T):
    """results: list of per-core dicts with 'out' [NP, 128, VS] -> [B, T, V] fp32."""
    NP = (T + 1) // 2
    full = np.concatenate([results[d]["out"] for d in range(R)], axis=2)  # [NP,128,VP]
    full = full.reshape(NP, 2, B, VP).transpose(2, 0, 1, 3).reshape(B, 2 * NP, VP)
    return np.ascontiguousarray(full[:, :T, :V])


def kernel(features, W_ih, W_hh, b_ih, b_hh, W_fc, b_fc, max_seq_len):
    T = int(max_seq_len)
    nc = get_nc(T)
    in_maps = make_in_maps(features, W_ih, W_hh, b_ih, b_hh, W_fc, b_fc)
    res = bass_utils.run_bass_kernel_spmd(nc, in_maps, core_ids=list(range(R)))
    return assemble(res.results, T)


# revision 20
# speedup vs baseline: 1.0026x; 1.0026x over previous
"""Trainium2 Bass kernel for nn_DecoderRNN (LSTM decoder + vocab projection).

Strategy (8 NeuronCores, SPMD):
  - Recurrence tensor-parallel: core d owns h-dims [128d, 128(d+1)). Each step it
    computes its 4x128 gate rows (transposed layout [gate_dim, batch]) with bf16
    matmuls accumulating in PSUM, applies sigmoid/tanh on ScalarE (bias fused),
    updates c/h on VectorE, and AllGathers the bf16 h^T shard ([128,64] per rank
    -> [1024,64]) so every core has the full hidden state for the next step.
  - fc projection vocab-parallel: core d owns 6400 vocab rows (V padded to 51200).
    W_fc^T (bf16) is SBUF-resident. Steps are batched in pairs so the stationary
    operand is [128k, 128m] (m = 2 steps x 64 batch) at full PE utilization,
    streaming W_fc^T in N=512 tiles. b_fc is added during the PSUM->SBUF drain
    on VectorE from an SBUF-resident [128, 6400] fp32 tile built once via K=1
    ones-matmuls.
  - x_proj = features @ W_ih^T is step-invariant: computed once on device, stored
    fp32, and injected into each step's PSUM group via an fp32 identity matmul.
"""

import numpy as np
import ml_dtypes

import concourse.bass as bass
import concourse.bacc as bacc
import concourse.tile as tile
import concourse.mybir as mybir
from concourse import bass_utils
from concourse.bass_interp import get_hw_module

BF = ml_dtypes.bfloat16
FP32 = mybir.dt.float32
BF16 = mybir.dt.bfloat16
AF = mybir.ActivationFunctionType

R = 8            # cores
B = 64           # batch
E = 512          # embed
H = 1024         # hidden
V = 50257        # vocab
VP = 51200       # padded vocab (multiple of 8*128)
VS = VP // R     # per-core vocab shard = 6400
NKH = H // 128   # 8 k-chunks over hidden
NKE = E // 128   # 4 k-chunks over embed
FC_N = 512       # fc free-dim tile
NFC = (VS + FC_N - 1) // FC_N  # 13 n-tiles (12x512 + 1x256)


def _emit(nc, tc, T, reps, wfcT, whhT, wihT, xT, bfc, bias, ident, out):
    NP = (T + 1) // 2  # step pairs
    with (
        tc.tile_pool(name="wpool", bufs=1) as wpool,
        tc.tile_pool(name="hpool", bufs=1) as hpool,
        tc.tile_pool(name="spool", bufs=3) as spool,
        tc.tile_pool(name="stage", bufs=6) as stage,
        tc.tile_pool(name="pg", bufs=1, space="PSUM") as pg_pool,
        tc.tile_pool(name="pf", bufs=4, space="PSUM") as pf_pool,
        tc.tile_pool(name="dram", bufs=2, space="DRAM") as dram,
    ):
        # ---- resident weights / constants (small DMAs first) ----
        wih_sb = wpool.tile([128, NKE, 512], BF16, name="wih_sb")
        nc.sync.dma_start(wih_sb[:], wihT.rearrange("(k p) n -> p k n", p=128))
        x_sb = wpool.tile([128, NKE, B], BF16, name="x_sb")
        nc.sync.dma_start(x_sb[:], xT.rearrange("(k p) n -> p k n", p=128))
        whh_sb = wpool.tile([128, NKH, 512], BF16, name="whh_sb")
        nc.sync.dma_start(whh_sb[:], whhT.rearrange("(k p) n -> p k n", p=128))
        bias_sb = wpool.tile([128, 4], FP32, name="bias_sb")
        nc.sync.dma_start(bias_sb[:], bias)
        id_sb = wpool.tile([128, 128], BF16, name="id_sb")
        nc.sync.dma_start(id_sb[:], ident)
        bfc_row = wpool.tile([1, VS], BF16, name="bfc_row")
        nc.sync.dma_start(bfc_row[:], bfc)
        ones_sb = wpool.tile([1, 128], BF16, name="ones_sb")
        nc.vector.memset(ones_sb[:], 1.0)
        # big W_fc^T load split by n-tile; DMAs are emitted inside the early
        # step slots so they don't delay the step-critical agi/hall DMAs
        wfc_sb = wpool.tile([128, NKH, VS], BF16, name="wfc_sb")
        wfcT_pkn = wfcT.rearrange("(k p) n -> p k n", p=128)

        def wfc_load(n_lo, n_hi):
            # issued from ScalarE (SWDGE) so the stream is not blocked behind
            # the Sync queue's hall-DMA wait on the first AllGather
            for n in range(n_lo, n_hi):
                nsz = min(FC_N, VS - n * FC_N)
                nc.scalar.dma_start(
                    wfc_sb[:, :, n * FC_N : n * FC_N + nsz],
                    wfcT_pkn[:, :, n * FC_N : n * FC_N + nsz],
                )

        bfc_sb = wpool.tile([128, VS], FP32, name="bfc_sb")

        # ---- x_proj^T [4 gate tiles][128, B], bf16 ----
        xproj_sb = wpool.tile([128, 4, B], BF16, name="xproj_sb")
        for m in range(4):
            px = pg_pool.tile([128, B], FP32, name="px", tag="pg0")
            for k in range(NKE):
                nc.tensor.matmul(
                    px[:], wih_sb[:, k, 128 * m : 128 * (m + 1)], x_sb[:, k, :],
                    start=(k == 0), stop=(k == NKE - 1),
                )
            nc.scalar.copy(xproj_sb[:, m, :], px[:])

        # persistent state
        hall = hpool.tile([128, NP, NKH, 2 * B], BF16, name="hall")
        cT = hpool.tile([128, B], FP32, name="cT")

        def fc_emit(p, n_lo, n_hi, msz):
            first_mm, last_mm = None, None
            for n in range(n_lo, n_hi):
                nsz = min(FC_N, VS - n * FC_N)
                pf = pf_pool.tile([128, FC_N], FP32, name="pf", tag="pf")
                for k in range(NKH):
                    i = nc.tensor.matmul(
                        pf[:msz, :nsz],
                        hall[:, p, k, :msz],
                        wfc_sb[:, k, n * FC_N : n * FC_N + nsz],
                        start=(k == 0), stop=(k == NKH - 1),
                    )
                    if first_mm is None:
                        first_mm = i
                    last_mm = i
                st = stage.tile([128, FC_N], FP32, name="st")
                nc.vector.tensor_add(
                    st[:msz, :nsz], pf[:msz, :nsz],
                    bfc_sb[:msz, n * FC_N : n * FC_N + nsz],
                )
                nc.sync.dma_start(
                    out[p, :msz, n * FC_N : n * FC_N + nsz], st[:msz, :nsz]
                )
            return first_mm, last_mm

        FC_SPLIT = 7  # n-tiles in the first half-chunk of a pair

        for rep in range(reps):
            nc.vector.memset(cT[:], 0.0)
            prev_filler_last = None
            for t in range(T):
                p, s = divmod(t, 2)
                # ---- gates^T in PSUM: 4 separate per-gate tiles [128, B] ----
                pg = [pg_pool.tile([128, B], FP32, name=f"pg{m}", tag=f"pg{m}") for m in range(4)]
                first_gates, last_gates = None, None
                for m in range(4):
                    i = nc.tensor.matmul(
                        pg[m][:], id_sb[:], xproj_sb[:, m, :],
                        start=True, stop=(t == 0),
                    )
                    if first_gates is None:
                        first_gates = i
                    last_gates = i
                    if t > 0:
                        pp, ss = divmod(t - 1, 2)
                        for k in range(NKH):
                            last_gates = nc.tensor.matmul(
                                pg[m][:],
                                whh_sb[:, k, 128 * m : 128 * (m + 1)],
                                hall[:, pp, k, B * ss : B * (ss + 1)],
                                start=False, stop=(k == NKH - 1),
                            )
                # keep PE stream interleaved: this slot's gates run after the
                # previous slot's PE filler work
                if prev_filler_last is not None:
                    tile.add_dep_helper(
                        first_gates.ins, prev_filler_last.ins, sync=False,
                        reason="slot order: gates after previous slot's fc filler",
                    )
                # ---- activations + c/h update (ordered for short critical path) ----
                i_s = spool.tile([128, B], FP32, name="i_s")
                f_s = spool.tile([128, B], FP32, name="f_s")
                g_t = spool.tile([128, B], FP32, name="g_t")
                o_s = spool.tile([128, B], FP32, name="o_s")
                t1 = spool.tile([128, B], FP32, name="t1")
                t2 = spool.tile([128, B], FP32, name="t2")
                tc_t = spool.tile([128, B], FP32, name="tc_t")
                h_bf = spool.tile([128, B], BF16, name="h_bf")
                nc.scalar.activation(i_s[:], pg[0][:], AF.Sigmoid, bias=bias_sb[:, 0:1])
                nc.scalar.activation(f_s[:], pg[1][:], AF.Sigmoid, bias=bias_sb[:, 1:2])
                nc.vector.tensor_mul(t2[:], f_s[:], cT[:])
                nc.scalar.activation(g_t[:], pg[2][:], AF.Tanh, bias=bias_sb[:, 2:3])
                nc.vector.tensor_mul(t1[:], i_s[:], g_t[:])
                nc.vector.tensor_add(cT[:], t1[:], t2[:])
                nc.scalar.activation(o_s[:], pg[3][:], AF.Sigmoid, bias=bias_sb[:, 3:4])
                nc.scalar.activation(tc_t[:], cT[:], AF.Tanh)
                nc.vector.tensor_mul(h_bf[:], o_s[:], tc_t[:])
                # ---- AllGather h^T shard -> full h^T (high priority: these
                # DMAs must not queue behind fc output stores) ----
                agi = dram.tile([128, B], BF16, name=f"agi{t}", tag=f"agi{t}")
                ago = dram.tile([H, B], BF16, name=f"ago{t}", tag=f"ago{t}")
                ago_pkn = ago.rearrange("(k p) n -> p k n", p=128)
                with tc.high_priority():
                    nc.sync.dma_start(agi[:], h_bf[:])
                    nc.gpsimd.collective_compute(
                        "AllGather",
                        mybir.AluOpType.bypass,
                        ins=[agi.opt()],
                        outs=[ago.opt()],
                        replica_groups=[list(range(R))],
                    )
                    # split so the first gate matmuls can start on the first half
                    nc.sync.dma_start(
                        hall[:, p, 0:4, B * s : B * (s + 1)], ago_pkn[:, 0:4, :]
                    )
                    nc.sync.dma_start(
                        hall[:, p, 4:8, B * s : B * (s + 1)], ago_pkn[:, 4:8, :]
                    )
                # W_fc^T stream: spread over the first slots, after this
                # slot's critical DMAs
                if rep == 0 and t == 0:
                    # the whole W_fc^T stream fits inside the first AllGather's
                    # launch-skew wait
                    wfc_load(0, NFC)
                # ---- PE filler for this slot's AG flight ----
                first_fill, last_fill = None, None
                if t == 0 and rep == 0:
                    # b_fc broadcast via K=1 ones-matmuls fills slot 0
                    for n in range(NFC):
                        nsz = min(FC_N, VS - n * FC_N)
                        pb = pf_pool.tile([128, FC_N], FP32, name="pb", tag="pf")
                        i = nc.tensor.matmul(
                            pb[:, :nsz], ones_sb[:], bfc_row[:, n * FC_N : n * FC_N + nsz],
                            start=True, stop=True,
                        )
                        if first_fill is None:
                            first_fill = i
                        last_fill = i
                        nc.vector.tensor_copy(
                            bfc_sb[:, n * FC_N : n * FC_N + nsz], pb[:, :nsz]
                        )
                elif t >= 2:
                    # fc chunk for a pair whose data landed at step t-1
                    q, half = divmod(t - 2, 2)
                    if half == 0:
                        first_fill, last_fill = fc_emit(q, 0, FC_SPLIT, 2 * B)
                    else:
                        first_fill, last_fill = fc_emit(q, FC_SPLIT, NFC, 2 * B)
                if first_fill is not None:
                    tile.add_dep_helper(
                        first_fill.ins, last_gates.ins, sync=False,
                        reason="slot order: fc filler after this slot's gates",
                    )
                    prev_filler_last = last_fill
                else:
                    prev_filler_last = last_gates
            # ---- tail: last pair(s) not covered in-loop ----
            fc_emit(NP - 1, 0, NFC, B if (T % 2) else 2 * B)


def build(T, reps=1):
    nc = bacc.Bacc(
        "TRN2",
        target_bir_lowering=False,
        debug=False,
        enable_asserts=False,
        num_devices=R,
    )
    NP = (T + 1) // 2
    wfcT = nc.dram_tensor("wfcT", [H, VS], BF16, kind="ExternalInput").ap()
    whhT = nc.dram_tensor("whhT", [H, 512], BF16, kind="ExternalInput").ap()
    wihT = nc.dram_tensor("wihT", [E, 512], BF16, kind="ExternalInput").ap()
    xT = nc.dram_tensor("xT", [E, B], BF16, kind="ExternalInput").ap()
    bfc = nc.dram_tensor("bfc", [1, VS], BF16, kind="ExternalInput").ap()
    bias = nc.dram_tensor("bias", [128, 4], FP32, kind="ExternalInput").ap()
    ident = nc.dram_tensor("ident", [128, 128], BF16, kind="ExternalInput").ap()
    out = nc.dram_tensor("out", [NP, 128, VS], FP32, kind="ExternalOutput").ap()

    with tile.TileContext(nc) as tc:
        _emit(nc, tc, T, reps, wfcT, whhT, wihT, xT, bfc, bias, ident, out)
    nc.compile()
    nc.m = get_hw_module(nc.m)
    return nc


_NC_CACHE = {}


def get_nc(T, reps=1):
    key = (T, reps)
    if key not in _NC_CACHE:
        _NC_CACHE[key] = build(T, reps)
    return _NC_CACHE[key]


def make_in_maps(features, W_ih, W_hh, b_ih, b_hh, W_fc, b_fc):
    features = np.asarray(features, np.float32)
    W_ih = np.asarray(W_ih, np.float32)
    W_hh = np.asarray(W_hh, np.float32)
    W_fc = np.asarray(W_fc, np.float32)
    b = np.asarray(b_ih, np.float32) + np.asarray(b_hh, np.float32)
    b_fc = np.asarray(b_fc, np.float32)

    xT_np = np.ascontiguousarray(features.T).astype(BF)
    ident_np = np.eye(128, dtype=np.float32).astype(BF)
    W_fc_pad = np.zeros((VP, H), np.float32)
    W_fc_pad[:V] = W_fc
    bfc_pad = np.zeros((VP,), np.float32)
    bfc_pad[:V] = b_fc

    in_maps = []
    for d in range(R):
        gsel = np.concatenate(
            [np.arange(g * H + d * 128, g * H + (d + 1) * 128) for g in range(4)]
        )
        whhT_np = np.ascontiguousarray(W_hh[gsel].T).astype(BF)
        wihT_np = np.ascontiguousarray(W_ih[gsel].T).astype(BF)
        bias_np = np.ascontiguousarray(b[gsel].reshape(4, 128).T)
        wfcT_np = np.ascontiguousarray(W_fc_pad[d * VS : (d + 1) * VS].T).astype(BF)
        bfc_np = bfc_pad[d * VS : (d + 1) * VS].reshape(1, VS).astype(BF)
        in_maps.append(
            {
                "wfcT": wfcT_np,
                "whhT": whhT_np,
                "wihT": wihT_np,
                "xT": xT_np,
                "bfc": bfc_np,
                "bias": bias_np,
                "ident": ident_np,
            }
        )
    return in_maps


def assemble(results, T):
    """results: list of per-core dicts with 'out' [NP, 128, VS] -> [B, T, V] fp32."""
    NP = (T + 1) // 2
    full = np.concatenate([results[d]["out"] for d in range(R)], axis=2)  # [NP,128,VP]
    full = full.reshape(NP, 2, B, VP).transpose(2, 0, 1, 3).reshape(B, 2 * NP, VP)
    return np.ascontiguousarray(full[:, :T, :V])


def kernel(features, W_ih, W_hh, b_ih, b_hh, W_fc, b_fc, max_seq_len):
    T = int(max_seq_len)
    nc = get_nc(T)
    in_maps = make_in_maps(features, W_ih, W_hh, b_ih, b_hh, W_fc, b_fc)
    res = bass_utils.run_bass_kernel_spmd(nc, in_maps, core_ids=list(range(R)))
    return assemble(res.results, T)
